# revision 1
# baseline (speedup 1.0000x reference)
"""Trainium2 Bass kernel for a 16-head dense attention block (B=1, S=2048, D=2048).

Sharding: 2 heads per core across 8 cores (tensor parallel on heads).
Key observation: the reference's (deliberate) transpose(2,3)+reshape before the
output projection makes output rows [h*128:(h+1)*128) depend ONLY on head h, so
per-core outputs are disjoint row blocks -> host-side concat, no collectives.

Per-core pipeline (all matmuls float32r, full PE rate at N>=256):
  1. Q^T,K^T [dh,S] and V [S,dh] projections from host-pretransposed x^T,
     processed per 512-wide s-block with full 16-d-tile PSUM accumulation.
     RoPE is applied per s-block right after each Q/K bank lands:
     q' = C*q + Sg*pairswap(q), pair-swap via SBUF->SBUF DMA, 1/sqrt(dh)
     folded into Q's rope constants.
  2. Attention in the transposed domain: S^T[k,q] = K@Q^T with the additive
     mask preloaded into PSUM via an identity matmul; exp on ScalarE
     (PSUM->SBUF, f32r out, no max subtraction -- inputs are bounded);
     O^T[dh,q] and row-sums (ones-matmul) accumulate over k tiles on PE.
     Mask blocks are classified host-side: all-(-inf) blocks are skipped
     entirely (exp underflows to exactly 0), all-zero blocks skip the
     mask preload. The causal mask reduces to 62.5% of blocks.
  3. Per-q-group epilogue: rowsums -> reciprocal (via DRAM-bounce
     redistribution), O^T -> O PE-transpose with the 1/rowsum scale fused
     into the PSUM->SBUF ACT copy.
  4. Output projection R_h = O_h^T @ wo^T accumulated over j tiles, with
     deep wo^T prefetch.
"""

import math

import numpy as np

S = 2048
D = 2048
H = 16
DH = 128
N_CORES = 8
HPC = H // N_CORES          # heads per core
NH = HPC * DH               # per-core head rows (256)
P = 128
QG = 512                    # q/s-group width
NQG = S // QG               # 4
NKT = S // P                # 16 k tiles
NDT = D // P                # 16 d tiles

SKIP, NOMASK, MASKED = 0, 1, 2

_CACHE = {}


def _build(block_kind):
    """block_kind: tuple of NQG tuples of NKT ints (SKIP/NOMASK/MASKED)."""
    import os
    import concourse.tile as tile
    from concourse import bacc, mybir

    B = lambda k, d: int(os.environ.get(k, d))
    f32 = mybir.dt.float32
    f32r = mybir.dt.float32r
    EXP = mybir.ActivationFunctionType.Exp

    nc = bacc.Bacc("TRN2", target_bir_lowering=False, debug=False,
                   num_devices=N_CORES)

    xT = nc.dram_tensor("xT", [D, S], f32r, kind="ExternalInput").ap()
    wqT = nc.dram_tensor("wqT", [D, NH], f32r, kind="ExternalInput").ap()
    wkT = nc.dram_tensor("wkT", [D, NH], f32r, kind="ExternalInput").ap()
    wvT = nc.dram_tensor("wvT", [D, NH], f32r, kind="ExternalInput").ap()
    maskT = nc.dram_tensor("maskT", [S, S], f32, kind="ExternalInput").ap()
    woT = nc.dram_tensor("woT", [S, D], f32r, kind="ExternalInput").ap()
    cq = nc.dram_tensor("cq", [DH, S], f32, kind="ExternalInput").ap()
    sq = nc.dram_tensor("sq", [DH, S], f32, kind="ExternalInput").ap()
    ck = nc.dram_tensor("ck", [DH, S], f32, kind="ExternalInput").ap()
    sk = nc.dram_tensor("sk", [DH, S], f32, kind="ExternalInput").ap()
    ident = nc.dram_tensor("ident", [P, P], f32r, kind="ExternalInput").ap()
    ones = nc.dram_tensor("ones", [P, 1], f32r, kind="ExternalInput").ap()
    out = nc.dram_tensor("out", [NH, D], f32, kind="ExternalOutput").ap()
    sums_dram = [nc.dram_tensor(f"sums_scratch{h}", [S], f32).ap()
                 for h in range(HPC)]

    with tile.TileContext(nc) as tc:
        with tc.tile_pool(name="consts", bufs=1) as consts, \
             tc.tile_pool(name="qkv", bufs=1) as qkv:

            id_t = consts.tile([P, P], f32r, tag="ident")
            nc.scalar.dma_start(id_t[:], ident[:])
            ones_t = consts.tile([P, 1], f32r, tag="ones")
            nc.scalar.dma_start(ones_t[:], ones[:])

            qt = [qkv.tile([P, S], f32, tag=f"qt{h}", name=f"qt{h}")
                  for h in range(HPC)]
            kt_ = [qkv.tile([P, S], f32, tag=f"kt{h}", name=f"kt{h}")
                   for h in range(HPC)]
            vt = qkv.tile([P, NKT, NH], f32, tag="v")    # [k-part, ktile, n]

            # ------------- phase 1+2: QKV projections + RoPE -------------
            xT_v = xT.rearrange("(t p) s -> t p s", p=P)       # [16,128,S]
            wT_v = {"q": wqT.rearrange("(t p) n -> t p n", p=P),
                    "k": wkT.rearrange("(t p) n -> t p n", p=P),
                    "v": wvT.rearrange("(t p) n -> t p n", p=P)}

            with tc.tile_pool(name="proj_sb", bufs=1) as proj_sb, \
                 tc.tile_pool(name="rope_sb", bufs=B("BR", 2)) as rope_sb, \
                 tc.tile_pool(name="ps_q", bufs=B("BQ", 3), space="PSUM") as ps_q, \
                 tc.tile_pool(name="ps_v", bufs=B("BV", 2), space="PSUM") as ps_v:
                wq_c = []
                xs0_c = []
                sl0 = slice(0, QG)
                for c in range(4):
                    cs = slice(c * 4, c * 4 + 4)
                    t = proj_sb.tile([P, 4, NH], f32r, tag=f"wq{c}",
                                     name=f"wq{c}")
                    nc.sync.dma_start(t[:],
                                      wT_v["q"][cs].rearrange("t p n -> p t n"))
                    wq_c.append(t)
                    # interleave g=0 x-chunks so the first matmul's pair of
                    # dependencies lands early on the sync FIFO
                    xc = proj_sb.tile([P, 4, QG], f32r, tag=f"xs{c}", bufs=2,
                                      name=f"xs{c}")
                    nc.sync.dma_start(
                        xc[:], xT_v[cs, :, sl0].rearrange("t p s -> p t s"))
                    xs0_c.append(xc)

                class _WQ:
                    def __getitem__(self, idx):
                        _, dt = idx[0], idx[1]
                        rest = idx[2:]
                        return wq_c[dt // 4][(slice(None), dt % 4) + rest]

                wts = {"q": _WQ()}
                for kind in ("k", "v"):
                    t = proj_sb.tile([P, NDT, NH], f32r, tag=f"w{kind}",
                                     name=f"w{kind}")
                    for c in range(4):
                        cs = slice(c * 4, c * 4 + 4)
                        nc.scalar.dma_start(
                            t[:, cs],
                            wT_v[kind][cs].rearrange("t p n -> p t n"))
                    wts[kind] = t
                rope_t = {}
                for nm, src in (("cq", cq), ("sq", sq), ("ck", ck), ("sk", sk)):
                    t = proj_sb.tile([DH, S], f32, tag=nm, name=nm)
                    nc.scalar.dma_start(t[:], src[:])
                    rope_t[nm] = t

                for g in range(NQG):
                    sl = slice(g * QG, (g + 1) * QG)
                    if g == 0:
                        xs_c = xs0_c
                    else:
                        xs_c = []
                        for c in range(4):
                            cs = slice(c * 4, c * 4 + 4)
                            xc = proj_sb.tile([P, 4, QG], f32r, tag=f"xs{c}",
                                              bufs=2, name=f"xs{c}")
                            nc.sync.dma_start(
                                xc[:],
                                xT_v[cs, :, sl].rearrange("t p s -> p t s"))
                            xs_c.append(xc)

                    _w = 4

                    class _XS:
                        def __init__(self, w):
                            self.w = w
                        def __getitem__(self, idx):
                            _, dt = idx[0], idx[1]
                            rest = idx[2:]
                            return xs_c[dt // self.w][
                                (slice(None), dt % self.w) + rest]
                    xs = _XS(_w)
                    for kind, dst, cn, sn in (("q", qt, "cq", "sq"),
                                              ("k", kt_, "ck", "sk")):
                        for h in range(HPC):
                            ps = ps_q.tile([P, QG], f32, tag="pq", name="pq")
                            for dt in range(NDT):
                                nc.tensor.matmul(
                                    ps[:], wts[kind][:, dt, h * P:(h + 1) * P],
                                    xs[:, dt], start=(dt == 0),
                                    stop=(dt == NDT - 1))
                            m = dst[h]
                            nc.vector.tensor_copy(m[:, sl].bitcast(f32r), ps[:])
                            # rope on this 512-wide slice
                            sw = rope_sb.tile([P, QG], f32, tag="sw", name="sw")
                            m_v = m[:, sl].rearrange("(j b) s -> j b s", b=2)
                            sw_v = sw.rearrange("(j b) s -> j b s", b=2)
                            nc.sync.dma_start(sw_v[:, 0], m_v[:, 1])
                            nc.sync.dma_start(sw_v[:, 1], m_v[:, 0])
                            t1 = rope_sb.tile([P, QG], f32, tag="t1", name="t1")
                            nc.vector.tensor_mul(t1[:], m[:, sl],
                                                 rope_t[cn][:, sl])
                            nc.vector.tensor_mul(sw[:], sw[:],
                                                 rope_t[sn][:, sl])
                            nc.vector.tensor_add(m[:, sl].bitcast(f32r),
                                                 t1[:], sw[:])
                    for st in range(g * 4, g * 4 + 4):
                        ps = ps_v.tile([P, NH], f32, tag="pv", name="pv")
                        lsl = slice((st % 4) * P, (st % 4) * P + P)
                        for dt in range(NDT):
                            nc.tensor.matmul(ps[:], xs[:, dt, lsl],
                                             wts["v"][:, dt], start=(dt == 0),
                                             stop=(dt == NDT - 1))
                        nc.vector.tensor_copy(vt[:, st].bitcast(f32r), ps[:])

            # ------------- phase 3: attention + per-g epilogue -----------
            maskT_v = maskT.rearrange("(t p) s -> t p s", p=P)
            with tc.tile_pool(name="att_persist", bufs=1) as att_p, \
                 tc.tile_pool(name="wo_sb", bufs=B("BW", 48)) as wo_sb:
                ot_sb = [att_p.tile([P, S], f32, tag=f"ot{h}", name=f"ot{h}")
                         for h in range(HPC)]
                sums_sb = [att_p.tile([1, S], f32, tag=f"sum{h}",
                                      name=f"sum{h}") for h in range(HPC)]
                rt = [att_p.tile([P, NKT], f32, tag=f"rt{h}", name=f"rt{h}")
                      for h in range(HPC)]

                att_ctx = (tc.tile_pool(name="att_sb", bufs=B("BA", 4)),
                           tc.tile_pool(name="ps_sc", bufs=B("BS", 3), space="PSUM"),
                           tc.tile_pool(name="ps_o", bufs=B("BO", 2), space="PSUM"),
                           tc.tile_pool(name="ps_sum", bufs=B("BSM", 2), space="PSUM"),
                           tc.tile_pool(name="ps_tr", bufs=B("BTR", 1), space="PSUM"))
                att_sb, ps_sc, ps_o, ps_sum, ps_tr = [
                    c.__enter__() for c in att_ctx]

                for g in range(NQG):
                    sl = slice(g * QG, (g + 1) * QG)
                    kinds = block_kind[g]
                    active = [kt for kt in range(NKT)
                              if kinds[kt][0] != SKIP]
                    first, last = active[0], active[-1]
                    pso = [ps_o.tile([P, QG], f32, tag="pso", name="pso")
                           for _ in range(HPC)]
                    pss = [ps_sum.tile([1, QG], f32, tag="pss", name="pss")
                           for _ in range(HPC)]
                    for kt in active:
                        masked, off = kinds[kt]
                        masked = masked == MASKED
                        if kt == first:
                            off = 0  # start=True must cover all PSUM columns
                        nw = QG - off
                        qsl = slice(g * QG + off, (g + 1) * QG)
                        osl = slice(off, QG)
                        if masked:
                            mt = att_sb.tile([P, QG], f32, tag="mask",
                                             name="mt")
                            nc.sync.dma_start(mt[:, osl],
                                              maskT_v[kt][:, qsl])
                        ksl = slice(kt * P, (kt + 1) * P)
                        for h in range(HPC):
                            ps = ps_sc.tile([P, QG], f32, tag="sc", name="sc")
                            nc.tensor.matmul(ps[:, osl],
                                             kt_[h][:, ksl].bitcast(f32r),
                                             qt[h][:, qsl].bitcast(f32r),
                                             start=True, stop=True)
                            pt = att_sb.tile([P, QG], f32r, tag="pt",
                                             name="pt")
                            if masked:
                                sm = att_sb.tile([P, QG], f32, tag="sm",
                                                 name="sm")
                                nc.vector.tensor_add(sm[:, osl], ps[:, osl],
                                                     mt[:, osl])
                                nc.scalar.activation(pt[:, osl], sm[:, osl],
                                                     EXP)
                            else:
                                nc.scalar.activation(pt[:, osl], ps[:, osl],
                                                     EXP)
                            nc.tensor.matmul(
                                pso[h][:, osl],
                                vt[:, kt, h * P:(h + 1) * P].bitcast(f32r),
                                pt[:, osl], start=(kt == first),
                                stop=(kt == last))
                            nc.tensor.matmul(pss[h][:, osl], ones_t[:],
                                             pt[:, osl],
                                             start=(kt == first),
                                             stop=(kt == last))
                    # per-g epilogue: sums -> rt slice, O^T -> O w/ scaling
                    for h in range(HPC):
                        nc.vector.tensor_copy(ot_sb[h][:, sl].bitcast(f32r),
                                              pso[h][:])
                        nc.scalar.copy(sums_sb[h][:, sl], pss[h][:])
                        nc.sync.dma_start(sums_dram[h][sl], sums_sb[h][:, sl])
                        nc.sync.dma_start(
                            rt[h][:, g * 4:g * 4 + 4],
                            sums_dram[h][sl].rearrange("(t p) -> p t", p=P))
                        nc.vector.reciprocal(rt[h][:, g * 4:g * 4 + 4],
                                             rt[h][:, g * 4:g * 4 + 4])
                        for jt in range(g * 4, g * 4 + 4):
                            jsl = slice(jt * P, (jt + 1) * P)
                            pst = ps_tr.tile([P, P], f32, tag="tr", name="tr")
                            nc.tensor.transpose(pst[:], ot_sb[h][:, jsl],
                                                id_t[:].bitcast(f32))
                            nc.scalar.mul(ot_sb[h][:, jsl].bitcast(f32r),
                                          pst[:], rt[h][:, jt:jt + 1])

                for c in reversed(att_ctx):
                    c.__exit__(None, None, None)

                # ------------- phase 4: output projection ----------------
                o_sb = ot_sb
                woT_v = woT.rearrange("(t p) m -> t p m", p=P)
                with tc.tile_pool(name="r_sb", bufs=4) as r_sb, \
                     tc.tile_pool(name="ps_r", bufs=4, space="PSUM") as ps_r:
                    for mg in range(NQG):
                        msl = slice(mg * QG, (mg + 1) * QG)
                        psr = [ps_r.tile([P, QG], f32, tag="psr", name="psr")
                               for _ in range(HPC)]
                        for jt in range(NKT):
                            wt = wo_sb.tile([P, QG], f32r, tag="wo", name="wt")
                            nc.sync.dma_start(wt[:], woT_v[jt][:, msl])
                            jsl = slice(jt * P, (jt + 1) * P)
                            for h in range(HPC):
                                nc.tensor.matmul(
                                    psr[h][:], o_sb[h][:, jsl].bitcast(f32r),
                                    wt[:], start=(jt == 0),
                                    stop=(jt == NKT - 1))
                        for h in range(HPC):
                            rs = r_sb.tile([P, QG], f32, tag="rs", name="rs")
                            nc.vector.tensor_copy(rs[:], psr[h][:])
                            nc.sync.dma_start(out[h * P:(h + 1) * P, msl],
                                              rs[:])

    nc.compile()
    return nc


def _classify_mask(maskT):
    """Per (g, kt) block: SKIP if exp(s+m) underflows to 0 for the whole
    block, NOMASK if the block is exactly zero, else MASKED."""
    kinds = []
    for g in range(NQG):
        row = []
        for kt in range(NKT):
            blk = maskT[kt * P:(kt + 1) * P, g * QG:(g + 1) * QG]
            if np.all(blk <= -1e5):
                row.append((SKIP, 0))
            elif not blk.any():
                row.append((NOMASK, 0))
            else:
                off = 0
                while (off + P <= QG - 2 * P
                       and np.all(blk[:, off:off + P] <= -1e5)):
                    off += P
                row.append((MASKED, off))
        kinds.append(tuple(row))
    return tuple(kinds)


def _get_nc(block_kind):
    key = ("nc", block_kind)
    if key not in _CACHE:
        _CACHE[key] = _build(block_kind)
    return _CACHE[key]


def _prep_inputs(x, freqs_cos, freqs_sin, mask, wq, wk, wv, wo):
    f = np.float32
    x = np.asarray(x, f).reshape(S, D)
    mask = np.asarray(mask, f).reshape(S, S)
    wq, wk, wv, wo = (np.asarray(w, f) for w in (wq, wk, wv, wo))
    cos = np.asarray(freqs_cos, f)
    sin = np.asarray(freqs_sin, f)

    xT = np.ascontiguousarray(x.T)
    maskT = np.ascontiguousarray(mask.T)
    woT = np.ascontiguousarray(wo.T)

    C = np.repeat(cos.T, 2, axis=0)          # [128, S], rows 2j,2j+1 = cos_j
    Sg = np.repeat(sin.T, 2, axis=0)
    Sg[0::2] *= -1.0                          # even rows: -sin, odd: +sin
    scale = 1.0 / math.sqrt(DH)
    common = {
        "xT": xT, "maskT": maskT, "woT": woT,
        "cq": np.ascontiguousarray(C * scale),
        "sq": np.ascontiguousarray(Sg * scale),
        "ck": C, "sk": Sg,
        "ident": np.eye(P, dtype=f),
        "ones": np.ones((P, 1), f),
    }
    in_maps = []
    for c in range(N_CORES):
        rows = slice(c * NH, (c + 1) * NH)
        in_maps.append(dict(
            common,
            wqT=np.ascontiguousarray(wq[rows].T),
            wkT=np.ascontiguousarray(wk[rows].T),
            wvT=np.ascontiguousarray(wv[rows].T),
        ))
    return in_maps


def kernel(x, freqs_cos, freqs_sin, mask, wq, wk, wv, wo, start_pos):
    from concourse.bass_utils import run_bass_kernel_spmd

    in_maps = _prep_inputs(x, freqs_cos, freqs_sin, mask, wq, wk, wv, wo)
    nc = _get_nc(_classify_mask(in_maps[0]["maskT"]))
    res = run_bass_kernel_spmd(nc, in_maps, core_ids=list(range(N_CORES)))
    full = np.concatenate([res.results[c]["out"] for c in range(N_CORES)],
                          axis=0)
    return full.reshape(1, S, D).astype(np.float32)

